# revision 48
# baseline (speedup 1.0000x reference)
"""Trainium2 Bass kernel: RMSNorm + QKV + YaRN RoPE + sliding-window GQA attention
with sink logits + output projection + residual.

Sharding: data-parallel over batch (2) x tensor-parallel over KV-head pairs (4).
Each of the 8 cores computes, for one batch element and 2 of the 8 KV heads
(16 of the 64 Q heads), the fused block and a partial output projection.
The host sums the 4 partial projections per batch and adds bias + residual.

Layout/precision strategy (v2):
  - RMSNorm fully folded on host: x is pre-normalized (fp32) then cast bf16.
  - Weights (qkv/out) and activations (x/q/k/v/attn) in bf16; matmul moving
    operands >=512 wide run at 1 cycle/row either way, so bf16 only halves
    DMA traffic, not PE time.  All PSUM accumulation stays fp32.
  - Softmax sink term folded into the PV accumulation as a 1-partition
    matmul; mask injected via matmul from a staged mask against identity.
  - Single merged schedule: K/V (+ Q m0) matmuls chase the x DMA stream,
    attention groups interleave with the Q m4..m7 tiles, output projection
    chunk 0 interleaves with the last attention groups.
"""

import numpy as np
import ml_dtypes

import concourse.bass as bass
import concourse.tile as tile
from concourse import bacc, mybir
from concourse.bass_utils import run_bass_kernel_spmd

# problem constants
B, SEQ, HID = 2, 1024, 2880
NH, NKV, D = 64, 8, 64
HIDP = 2944            # 23 * 128
KT = HIDP // 128       # 23 hidden k-tiles
QKV_M = 10             # 1280 rows per core / 128
OUT_M = KT             # output hidden tiles (padded)
OUT_K = 8              # 1024 attn features / 128
NT = SEQ               # tokens per core
CH = 512               # matmul moving chunk
EPS = 1e-5
MASK_NEG = -100.0

F32 = mybir.dt.float32
F32R = mybir.dt.float32r
BF16 = mybir.dt.bfloat16
FP8 = mybir.dt.float8e4

PAIR_SWAP = [i ^ 1 for i in range(32)]


# ---------------------------------------------------------------- device code
def build_nc(reps=1, timing_mode=False, debug=False):
    nc = bacc.Bacc("TRN2", target_bir_lowering=False, debug=False)

    big = "Internal" if timing_mode else "ExternalInput"
    xt_d = nc.dram_tensor("xt", [128, KT * NT], BF16, kind=big)
    wqkv_d = nc.dram_tensor("wqkv", [QKV_M, 128, KT * 128], BF16, kind=big)
    wqc_d = nc.dram_tensor("wqc", [128, KT * 512], BF16, kind=big)
    bqkv_d = nc.dram_tensor("bqkv", [128, QKV_M], F32, kind="ExternalInput")
    wout_d = nc.dram_tensor("wout", [OUT_M, 128, OUT_K * 128], BF16, kind=big)
    cos_d = nc.dram_tensor("cos128", [128, NT], BF16, kind="ExternalInput")
    sin_d = nc.dram_tensor("sin128", [128, NT], BF16, kind="ExternalInput")
    maskt_d = nc.dram_tensor("maskt", [128, 256], F32R, kind="ExternalInput")
    identb_d = nc.dram_tensor("identb", [128, CH], F32R, kind="ExternalInput")
    sink_d = nc.dram_tensor("sinkrow", [1, 4 * CH], F32R, kind="ExternalInput")
    sinksel_d = nc.dram_tensor("sinksel", [1, 65], F32R, kind="ExternalInput")
    id_d = nc.dram_tensor("ident64", [128, 64], BF16, kind="ExternalInput")
    ones64_d = nc.dram_tensor("ones64", [1, 64], F32R, kind="ExternalInput")
    wo8_d = nc.dram_tensor("wo8", [128, OUT_M * 256], FP8, kind=big)
    maskdb_d = nc.dram_tensor("maskdb", [128, CH], BF16, kind="ExternalInput")
    masklb_d = nc.dram_tensor("masklb", [128, CH], BF16, kind="ExternalInput")
    out_d = nc.dram_tensor("out_t", [OUT_M, 128, NT], F32,
                           kind="Internal" if timing_mode else "ExternalOutput")
    dummy_d = (nc.dram_tensor("timing_out", [1, 2], F32, kind="ExternalOutput")
               if timing_mode else None)
    if debug:
        dbg_k = nc.dram_tensor("dbg_k", [128, NT], BF16, kind="ExternalOutput")
        dbg_v = nc.dram_tensor("dbg_v", [128, NT], BF16, kind="ExternalOutput")
        dbg_q = nc.dram_tensor("dbg_q", [128, 8 * NT], BF16, kind="ExternalOutput")
        dbg_a = nc.dram_tensor("dbg_a", [128, 8 * NT], BF16, kind="ExternalOutput")
        dbg_vt = nc.dram_tensor("dbg_vt", [128, 16 * 65], BF16, kind="ExternalOutput")

    mult = mybir.AluOpType.mult

    with tile.TileContext(nc) as tc:
      for rep in range(reps):
        with tc.tile_pool(name="singles", bufs=1) as singles:
            cos_sb = singles.tile([128, NT], BF16)
            sin_sb = singles.tile([128, NT], BF16)
            bq_sb = singles.tile([128, QKV_M], F32)
            sink_sb = singles.tile([1, 4 * CH], F32R)
            sinksel_sb = singles.tile([1, 65], F32R)
            id_sb = singles.tile([128, 64], BF16)
            ones64_sb = singles.tile([1, 64], F32R)
            wo8_sb = singles.tile([128, OUT_M * 256], FP8)
            attn8_sb = singles.tile([128, 2 * NT], FP8)
            maskdb_sb = singles.tile([128, CH], BF16)
            masklb_sb = singles.tile([128, CH], BF16)
            maskt_sb = singles.tile([128, 256], F32R)
            identb_sb = singles.tile([128, CH], F32R)

            q_sb = singles.tile([128, 8 * NT], BF16)
            k_sb = singles.tile([128, NT], BF16)
            v_sb = singles.tile([128, NT], BF16)
            vt_sb = singles.tile([128, 16 * 65], BF16)

            q_v = q_sb.rearrange("p (h t) -> p h t", t=NT)

            with (
                tc.tile_pool(name="xtp", bufs=1) as xtp,
                tc.tile_pool(name="wqp", bufs=4) as wqp,
                tc.tile_pool(name="ropep", bufs=2) as ropep,
            ):
                xt_sb = xtp.tile([128, KT * NT], BF16)

                # --- DMA issue: x stream on sync queue; weights on pool queue
                # (kept off the Act queue so epilogues never delay loads).
                # wq8/wq9 in per-k chunks interleave with the x stream so the
                # K/V chase can start immediately; wq0 chunks ride along so
                # the Q m0 matmuls can join the chase.
                wqc_sb = xtp.tile([128, KT * 512], BF16)
                wq_tiles = {m: wqp.tile([128, KT * 128], BF16, tag="wq",
                                        name=f"wqm{m}")
                            for m in range(2, 8)}

                def wc_dma(lo, hi):
                    nc.sync.dma_start(out=wqc_sb[:, lo * 512:hi * 512],
                                      in_=wqc_d[:, lo * 512:hi * 512])

                def x_dma(lo, hi):
                    nc.sync.dma_start(out=xt_sb[:, lo * NT:hi * NT],
                                      in_=xt_d[:, lo * NT:hi * NT])

                nc.sync.dma_start(out=xt_sb[:, 0:CH], in_=xt_d[:, 0:CH])
                nc.sync.dma_start(out=wqc_sb[:, 0:512], in_=wqc_d[:, 0:512])
                nc.sync.dma_start(out=xt_sb[:, CH:NT], in_=xt_d[:, CH:NT])
                wc_dma(1, 4)
                x_dma(1, 2)
                x_dma(2, 5)
                wc_dma(4, 8)
                x_dma(5, 9)
                wc_dma(8, 13)
                def gate(sb, col):
                    # 1-element copy whose input arrives late: blocks the
                    # subsequent DMA write of sb from being hoisted early
                    nc.vector.tensor_copy(out=sb[0:1, 0:1],
                                          in_=xt_sb[0:1, col:col + 1])

                for sb in (cos_sb, sin_sb, bq_sb):
                    gate(sb, 8 * NT)
                nc.sync.dma_start(out=cos_sb, in_=cos_d[:, :])
                nc.sync.dma_start(out=sin_sb, in_=sin_d[:, :])
                nc.sync.dma_start(out=bq_sb, in_=bqkv_d[:, :])
                x_dma(9, 14)
                wc_dma(13, 18)
                x_dma(14, 19)
                wc_dma(18, KT)
                x_dma(19, KT)
                for sb in (sink_sb, sinksel_sb, maskt_sb, identb_sb, id_sb,
                           ones64_sb, maskdb_sb, masklb_sb):
                    gate(sb, 13 * NT)
                nc.sync.dma_start(out=sink_sb, in_=sink_d[:, :])
                nc.sync.dma_start(out=sinksel_sb, in_=sinksel_d[:, :])
                nc.sync.dma_start(out=maskt_sb, in_=maskt_d[:, :])
                nc.sync.dma_start(out=identb_sb, in_=identb_d[:, :])
                nc.sync.dma_start(out=id_sb, in_=id_d[:, :])
                nc.sync.dma_start(out=ones64_sb, in_=ones64_d[:, :])
                gate(wo8_sb, 13 * NT)
                nc.sync.dma_start(out=wo8_sb, in_=wo8_d[:, :])
                nc.sync.dma_start(out=maskdb_sb, in_=maskdb_d[:, :])
                nc.sync.dma_start(out=masklb_sb, in_=masklb_d[:, :])
                for m in range(2, 8):
                    gate(wq_tiles[m], (8 + m) * NT)
                    nc.sync.dma_start(out=wq_tiles[m], in_=wqkv_d[m, :, :])
                for t in range(16):
                    nc.vector.memset(vt_sb[:, t * 65 + 64:t * 65 + 65], 1.0)

                def qkv_dest(m):
                    if m == 8:
                        return k_sb
                    if m == 9:
                        return v_sb
                    return q_sb[:, m * NT:(m + 1) * NT]

                def qkv_epilogue(m, ps_cs):
                    dest = qkv_dest(m)
                    for c, ps in enumerate(ps_cs):
                        nc.scalar.activation(
                            out=dest[:, c * CH:(c + 1) * CH], in_=ps,
                            func=mybir.ActivationFunctionType.Identity,
                            bias=bq_sb[:, m:m + 1], scale=1.0,
                        )

                def rope(dest, c=None):
                    # per-chunk so the first 512 tokens unblock attention
                    # groups before the second half's DVE chain drains
                    cs = (0, 1) if c is None else (c,)
                    for cc in cs:
                        sl = slice(cc * CH, (cc + 1) * CH)
                        sh = ropep.tile([128, CH], BF16, tag="rope")
                        nc.vector.stream_shuffle(out=sh, in_=dest[:, sl],
                                                 mask=PAIR_SWAP)
                        nc.vector.tensor_mul(sh, sh, sin_sb[:, sl])
                        nc.vector.tensor_mul(dest[:, sl], dest[:, sl],
                                             cos_sb[:, sl])
                        nc.vector.tensor_add(dest[:, sl], dest[:, sl], sh)

                # --- chase: K/V and Q m0 matmuls follow the x stream
                kv_ctx = tc.tile_pool(name="ps_kv", bufs=2, space="PSUM")
                ps_kv = kv_ctx.__enter__()
                ps8 = ps_kv.tile([128, NT], F32, tag="kv", name="mm8")
                ps9 = ps_kv.tile([128, NT], F32, tag="kv", name="mm9")
                # warm-up matmuls on a memset tile keep the PE clock ramping
                # while the first x/weight DMAs land
                warm = xtp.tile([128, CH], BF16)
                nc.gpsimd.memset(warm, 0.0)
                for i in range(8):
                    nc.tensor.matmul(
                        ps8[:, 0:CH], warm[:, 0:128], warm,
                        start=(i == 0), stop=(i == 7),
                    )
                mm_ctx = tc.tile_pool(name="ps_mm", bufs=4, space="PSUM",
                                      side="right")
                ps_mm = mm_ctx.__enter__()
                ps0 = [ps_mm.tile([128, CH], F32, tag="mm", name=f"mm0c{c}")
                       for c in range(2)]
                ps1 = [ps_mm.tile([128, CH], F32, tag="mm", name=f"mm1c{c}")
                       for c in range(2)]
                for k in range(KT):
                    for j, psx in enumerate((ps8, ps9, ps0, ps1)):
                        lhsT = wqc_sb[:, k * 512 + j * 128:k * 512 + (j + 1) * 128]
                        for c in range(2):
                            out = (psx[:, c * CH:(c + 1) * CH] if j < 2
                                   else psx[c])
                            nc.tensor.matmul(
                                out,
                                lhsT,
                                xt_sb[:, k * NT + c * CH: k * NT + (c + 1) * CH],
                                start=(k == 0),
                                stop=(k == KT - 1),
                            )

                qkv_epilogue(0, ps0)
                rope(qkv_dest(0))
                qkv_epilogue(1, ps1)
                rope(qkv_dest(1))
                qkv_epilogue(8, (ps8[:, 0:CH], ps8[:, CH:NT]))
                rope(k_sb)
                qkv_epilogue(9, (ps9[:, 0:CH], ps9[:, CH:NT]))
                kv_ctx.__exit__(None, None, None)
                vt_ctx = tc.tile_pool(name="ps_vt", bufs=2, space="PSUM")
                ps_vt = vt_ctx.__enter__()
                for g in range(2):
                    for kt in range(8):
                        pst = ps_vt.tile([128, 64], BF16, tag="vt")
                        nc.tensor.matmul(
                            pst,
                            v_sb[g * 64:(g + 1) * 64, kt * 128:(kt + 1) * 128],
                            id_sb[g * 64:(g + 1) * 64, :],
                            is_transpose=True,
                            start=True, stop=True,
                        )
                        nc.vector.tensor_copy(
                            out=vt_sb[:, (g * 8 + kt) * 65:(g * 8 + kt) * 65 + 64],
                            in_=pst,
                        )
                vt_ctx.__exit__(None, None, None)

                def qkv_mms(m, wqx, inject=False):
                    ps_cs = []
                    for c in range(2):
                        ps = ps_mm.tile([128, CH], F32, tag="mm",
                                        name=f"mm{m}c{c}")
                        ps_cs.append(ps)
                        for k in range(KT):
                            lhsT = wqx[:, k * 128:(k + 1) * 128]
                            nc.tensor.matmul(
                                ps,
                                lhsT,
                                xt_sb[:, k * NT + c * CH: k * NT + (c + 1) * CH],
                                start=(k == 0),
                                stop=(k == KT - 1),
                            )
                            if inject and k % 4 == 3:
                                att_step(copy_mask=True)
                    return ps_cs

                def q_tile(m, inject=False):
                    ps_cs = qkv_mms(m, wq_tiles.pop(m), inject)
                    dest = qkv_dest(m)
                    for c, ps in enumerate(ps_cs):
                        nc.scalar.activation(
                            out=dest[:, c * CH:(c + 1) * CH], in_=ps,
                            func=mybir.ActivationFunctionType.Identity,
                            bias=bq_sb[:, m:m + 1], scale=1.0,
                        )
                        rope(dest, c)

                for m in range(2, 4):
                    q_tile(m)

                # --- attention machinery (shares the window with Q m4..m7)
                att_ctx = [
                    tc.tile_pool(name="attnp", bufs=1),
                    tc.tile_pool(name="wexp", bufs=5),
                    tc.tile_pool(name="dnp", bufs=6),
                ]
                attnp, wexp, dnp = [c.__enter__() for c in att_ctx]
                att1_ctx = [
                    tc.tile_pool(name="ps_att", bufs=2, space="PSUM"),
                    tc.tile_pool(name="ps_pv", bufs=2, space="PSUM"),
                ]
                cur_pools = [[c.__enter__() for c in att1_ctx] + [None]]

                attn_sb = attnp.tile([128, 8 * NT], BF16)
                a_v = attn_sb.rearrange("p (h t) -> p h t", t=NT)
                a8_v = attn8_sb.rearrange("p (h t) -> p h t", t=NT)
                wo8_v = wo8_sb.rearrange("p (m two f) -> p m two f",
                                         two=2, f=128)

                def attn_A(qt, a, g, copy_mask=False):
                    # copy_mask=True: skip the additive mask matmul; apply a
                    # binary mask multiplicatively on the exp output (DVE,
                    # bf16 2x).  Masked logits share the live logits' scale,
                    # so exp cannot overflow.
                    ps_att, ps_pv = cur_pools[0][:2]
                    prng = slice(g * 64, (g + 1) * 64)
                    kts = [qt] if qt == 0 else [qt - 1, qt]
                    rhs_q = q_v[prng, 4 * a:4 * a + 4, qt * 128:(qt + 1) * 128]
                    ws = []
                    for kt in kts:
                        psl = ps_att.tile([128, CH], F32, tag="l")
                        sel = 0 if kt == qt else 1
                        if copy_mask:
                            nc.tensor.matmul(
                                psl,
                                k_sb[prng, kt * 128:(kt + 1) * 128],
                                rhs_q,
                                start=True, stop=True,
                            )
                        else:
                            nc.tensor.matmul(
                                psl,
                                maskt_sb[:, sel * 128:(sel + 1) * 128],
                                identb_sb,
                                start=True, stop=False,
                            )
                            nc.tensor.matmul(
                                psl,
                                k_sb[prng, kt * 128:(kt + 1) * 128],
                                rhs_q,
                                start=False, stop=True,
                            )
                        w = wexp.tile([128, CH], BF16, tag="w")
                        nc.scalar.activation(
                            out=w, in_=psl, func=mybir.ActivationFunctionType.Exp
                        )
                        if copy_mask:
                            nc.vector.tensor_mul(
                                w, w, maskdb_sb if sel == 0 else masklb_sb
                            )
                        ws.append((kt, w))
                    return (qt, a, g, ws)

                def attn_B(st, pe_bcast=False):
                    ps_att, ps_pv = cur_pools[0][:2]
                    qt, a, g, ws = st
                    prng = slice(g * 64, (g + 1) * 64)
                    pspv = ps_pv.tile([65, CH], F32, tag="pv")
                    so = (2 * g + a) * CH
                    nc.tensor.matmul(
                        pspv, sinksel_sb, sink_sb[0:1, so:so + CH],
                        start=True, stop=False,
                    )
                    for i, (kt, w) in enumerate(ws):
                        nc.tensor.matmul(
                            pspv,
                            vt_sb[:, (g * 8 + kt) * 65:(g * 8 + kt + 1) * 65],
                            w,
                            start=False,
                            stop=(i == len(ws) - 1),
                        )
                    dn = dnp.tile([1, CH], F32, tag="dn")
                    dnb = dnp.tile([64, CH], F32, tag="dnb")
                    if pe_bcast:
                        # half-width chains (head pairs) pipeline across
                        # DVE/Pool, halving the latency that gates the pspv
                        # ring in attention-dense windows
                        for h in range(2):
                            sl = slice(h * 256, (h + 1) * 256)
                            nc.vector.reciprocal(out=dn[0:1, sl],
                                                 in_=pspv[64:65, sl])
                            nc.gpsimd.partition_broadcast(dnb[:, sl],
                                                          dn[0:1, sl])
                            nc.vector.tensor_tensor(
                                out=a_v[prng, 4 * a + 2 * h:4 * a + 2 * h + 2,
                                        qt * 128:(qt + 1) * 128],
                                in0=pspv[0:64, sl],
                                in1=dnb[:, sl],
                                op=mult,
                            )
                    else:
                        nc.vector.reciprocal(out=dn, in_=pspv[64:65, :])
                        nc.gpsimd.partition_broadcast(dnb, dn)
                        nc.vector.tensor_tensor(
                            out=a_v[prng, 4 * a:4 * a + 4,
                                    qt * 128:(qt + 1) * 128],
                            in0=pspv[0:64, :],
                            in1=dnb,
                            op=mult,
                        )
                    if a == 0:
                        # heads 0-1 also in fp8 for the DoubleRow pair
                        nc.vector.tensor_copy(
                            out=a8_v[prng, 0:2, qt * 128:(qt + 1) * 128],
                            in_=a_v[prng, 0:2, qt * 128:(qt + 1) * 128],
                        )

                from collections import deque
                att_pending = deque()
                att_inflight = deque()

                def att_step(copy_mask=False, pe_bcast=False):
                    if len(att_inflight) >= 2 or (not att_pending
                                                  and att_inflight):
                        attn_B(att_inflight.popleft(), pe_bcast=pe_bcast)
                    elif att_pending:
                        att_inflight.append(
                            attn_A(*att_pending.popleft(),
                                   copy_mask=copy_mask))

                def att_drain():
                    while att_pending or att_inflight:
                        att_step()

                def attn_group(qt, a, g):
                    att_pending.append((qt, a, g))
                    att_step()
                    att_step()

                # W1: Q m4..m7 with ALL a=0 attention groups pipelined;
                # the a=1 qt0-3 groups queue up behind them (their q tiles
                # finish during W1)
                att_pending.extend((qt, 0, g) for qt in range(8)
                                   for g in range(2))
                for m in range(4, 8):
                    q_tile(m, inject=True)
                att_drain()
                att_pending.extend((qt, 1, g) for qt in range(4)
                                   for g in range(2))
                for c in reversed(att1_ctx):
                    c.__exit__(None, None, None)
                mm_ctx.__exit__(None, None, None)
                att2_ctx = [
                    tc.tile_pool(name="ps_att2", bufs=3, space="PSUM"),
                    tc.tile_pool(name="ps_pv2", bufs=3, space="PSUM"),
                ]
                cur_pools[0] = [c.__enter__() for c in att2_ctx]

                # W2: a=1 qt0-3, pipelined (must fully drain before the
                # chunk-0 projection reads their outputs: issue order defines
                # read/write ordering for the dependency pass)
                for _ in range(14):
                    att_step(pe_bcast=True)
                att_drain()

                # W3/W4: output projection; chunk 0 interleaves with the
                # remaining attention groups (a=1, qt 4..7)
                with (
                    tc.tile_pool(name="wop", bufs=3) as wop,
                    tc.tile_pool(name="otp", bufs=3) as otp,
                    tc.tile_pool(name="ps_o", bufs=2, space="PSUM") as ps_o,
                ):
                    def outproj_m(cs, m, inject=False):
                        wo_sb = wop.tile([128, (OUT_K - 2) * 128], BF16,
                                         tag="wo", name=f"wo{cs[0]}_{m}")
                        if cs[0] == 0 and m < 3:
                            nc.vector.tensor_copy(out=wo_sb[0:1, 0:1],
                                                  in_=attn_sb[0:1, 0:1])
                        nc.scalar.dma_start(out=wo_sb,
                                            in_=wout_d[m, :, 256:OUT_K * 128])
                        for c in cs:
                            ps = ps_o.tile([128, CH], F32, tag="o",
                                           name=f"o{c}_{m}")
                            nc.tensor.matmul(
                                ps,
                                wo8_v[:, m, :, :],
                                a8_v[:, 0:2, c * CH:(c + 1) * CH],
                                start=True, stop=False,
                                perf_mode=mybir.MatmulPerfMode.DoubleRow,
                            )
                            for k in range(2, OUT_K):
                                nc.tensor.matmul(
                                    ps,
                                    wo_sb[:, (k - 2) * 128:(k - 1) * 128],
                                    attn_sb[:, k * NT + c * CH: k * NT + (c + 1) * CH],
                                    start=False,
                                    stop=(k == OUT_K - 1),
                                )
                                if inject and k % 3 == 2:
                                    att_step(copy_mask=True, pe_bcast=True)
                            ot = otp.tile([128, CH], F32, tag="ot")
                            nc.scalar.copy(out=ot, in_=ps)
                            nc.sync.dma_start(
                                out=out_d[m, :, c * CH:(c + 1) * CH],
                                in_=ot,
                            )

                    # m 0..7: chunk-0 projection with the last attention
                    # groups pipelined in; m 8..22: both chunks per wo load;
                    # then chunk 1 for m 0..7 (wo reloaded).
                    att_pending.extend((qt, 1, g) for qt in range(4, 8)
                                       for g in range(2))
                    for m in range(8):
                        outproj_m((0,), m, inject=True)
                    att_drain()

                    for m in range(8, OUT_M):
                        outproj_m((0, 1), m)
                    for m in range(7):
                        outproj_m((1,), m)
                    # final tile: two half-width PSUM chains so the first
                    # half's copy+store overlap the second half's matmuls
                    wo_sb = wop.tile([128, (OUT_K - 2) * 128], BF16,
                                     tag="wo", name="wo1_7f")
                    nc.scalar.dma_start(out=wo_sb,
                                        in_=wout_d[7, :, 256:OUT_K * 128])
                    for h, q in ((0, nc.scalar), (1, nc.sync)):
                        psf = ps_o.tile([128, CH], F32, tag="o", name=f"oh{h}")
                        ps = psf[:, 0:CH // 2]
                        nc.tensor.matmul(
                            ps,
                            wo8_v[:, 7, :, :],
                            a8_v[:, 0:2, CH + h * 256:CH + (h + 1) * 256],
                            start=True, stop=False,
                            perf_mode=mybir.MatmulPerfMode.DoubleRow,
                        )
                        for k in range(2, OUT_K):
                            nc.tensor.matmul(
                                ps,
                                wo_sb[:, (k - 2) * 128:(k - 1) * 128],
                                attn_sb[:, k * NT + CH + h * 256:
                                        k * NT + CH + (h + 1) * 256],
                                start=False,
                                stop=(k == OUT_K - 1),
                            )
                        ot = otp.tile([128, CH // 2], F32, tag="oth")
                        if h == 0:
                            nc.scalar.copy(out=ot, in_=ps)
                        else:
                            nc.vector.tensor_copy(out=ot, in_=ps)
                        q.dma_start(
                            out=out_d[7, :, CH + h * 256:CH + (h + 1) * 256],
                            in_=ot,
                        )

                if debug:
                    for dram, sb in ((dbg_k, k_sb), (dbg_v, v_sb),
                                     (dbg_q, q_sb), (dbg_a, attn_sb),
                                     (dbg_vt, vt_sb)):
                        nc.sync.dma_start(out=dram[:, :], in_=sb)

                for c in reversed(att2_ctx):
                    c.__exit__(None, None, None)
                for c in reversed(att_ctx):
                    c.__exit__(None, None, None)

      if timing_mode:
          with tc.tile_pool(name="dummyp", bufs=1) as dummyp:
              dt_sb = dummyp.tile([1, 2], F32)
              nc.vector.memset(dt_sb, 1.0)
              nc.sync.dma_start(out=dummy_d[:, :], in_=dt_sb)

    nc.compile()
    return nc


# ---------------------------------------------------------------- host prep
def _rope_tables():
    # verbatim fp32 port of the reference YaRN cache
    steps = np.arange(0, 64, 2, dtype=np.float32)
    freq = np.power(np.float32(150000.0), steps / np.float32(64))
    conc = np.float32(0.1) * np.log(np.float32(32.0)) + 1.0
    d_half = np.float32(32.0)
    log_base = np.log(np.float32(150000.0))
    low = d_half * np.log(np.float32(4096) / (np.float32(32.0) * np.float32(2.0 * np.pi))) / log_base
    high = d_half * np.log(np.float32(4096) / (np.float32(1.0) * np.float32(2.0 * np.pi))) / log_base
    ramp = (np.arange(32, dtype=np.float32) - low) / (high - low)
    mask = 1.0 - np.clip(ramp, 0.0, 1.0)
    inv_freq = (1.0 / (np.float32(32.0) * freq)) * (1.0 - mask) + (1.0 / freq) * mask
    pos = np.arange(SEQ, dtype=np.float32)
    freqs = np.einsum("i,j->ij", pos, inv_freq.astype(np.float32))
    cos = (np.cos(freqs) * conc).astype(np.float32)  # (SEQ, 32)
    sin = (np.sin(freqs) * conc).astype(np.float32)
    return cos, sin


def _round_f32r(a):
    """Round-to-nearest onto the fp32r grid (low 12 mantissa bits zero)."""
    b = np.ascontiguousarray(a, np.float32).view(np.uint32)
    out = ((b + 0x800) & 0xFFFFF000).astype(np.uint32)
    return out.view(np.float32)


def _bf16(a):
    return np.ascontiguousarray(a.astype(ml_dtypes.bfloat16))


_ILV = np.empty(64, np.int64)
_ILV[0::2] = np.arange(32)
_ILV[1::2] = np.arange(32) + 32


def prep_inputs(x, norm_w, qkv_w, qkv_b, out_w, sinks):
    x = np.asarray(x, np.float32)
    norm_w = np.asarray(norm_w, np.float32)
    qkv_w = np.asarray(qkv_w, np.float32)
    qkv_b = np.asarray(qkv_b, np.float32)
    out_w = np.asarray(out_w, np.float32)
    sinks = np.asarray(sinks, np.float32)

    # host RMSNorm (norm_w folded into qkv_w below)
    rms = np.mean(x * x, axis=-1, keepdims=True, dtype=np.float32)
    xn = x * (1.0 / np.sqrt(rms + np.float32(EPS)))

    cos, sin = _rope_tables()
    cosT, sinT = cos.T, sin.T                      # (32, SEQ)
    cos64 = np.repeat(cosT, 2, axis=0)             # lo/hi both use cos_i
    sin64 = np.repeat(sinT, 2, axis=0).copy()
    sin64[0::2] *= -1.0                            # lo gets -sin
    cos128 = _bf16(np.concatenate([cos64, cos64], axis=0))
    sin128 = _bf16(np.concatenate([sin64, sin64], axis=0))

    i = np.arange(128)[:, None]
    j = np.arange(128)[None, :]
    maskd = np.where(i <= j, 0.0, MASK_NEG).astype(np.float32)
    maskl = np.where(i > j, 0.0, MASK_NEG).astype(np.float32)
    maskt = np.ascontiguousarray(np.concatenate([maskd.T, maskl.T], axis=1))
    identb = np.ascontiguousarray(np.tile(np.eye(128, dtype=np.float32), (1, 4)))
    sinksel = np.zeros((1, 65), np.float32)
    sinksel[0, 64] = 1.0
    eye = np.eye(64, dtype=np.float32)
    ident64 = _bf16(np.concatenate([eye, eye], axis=0))  # (128, 64)
    ones64 = np.ones((1, 64), np.float32)
    bind = np.where(i <= j, 1.0, 0.0).astype(np.float32)   # keep: diag block
    binl = np.where(i > j, 1.0, 0.0).astype(np.float32)    # keep: low block
    maskdb = _bf16(np.tile(bind, (1, 4)))
    masklb = _bf16(np.tile(binl, (1, 4)))

    w_eff = qkv_w * norm_w[None, :]
    b_eff = qkv_b.copy()
    w_eff[:NH * D] *= 0.125
    b_eff[:NH * D] *= 0.125

    in_maps = []
    for c in range(8):
        b, g2 = divmod(c, 4)
        # Q m-tile m holds heads (16*g2+m) [partitions 0:64] and (16*g2+8+m)
        # [partitions 64:128], rope-pair interleaved within each head.
        qheads = np.empty(16, np.int64)
        qheads[0::2] = 16 * g2 + np.arange(8)        # g=0 heads, even slots
        qheads[1::2] = 16 * g2 + 8 + np.arange(8)    # g=1 heads, odd slots
        qrows = (qheads[:, None] * D + _ILV[None, :]).reshape(-1)
        krows = NH * D + np.arange(2 * g2 * D, 2 * (g2 + 1) * D)
        vrows = (NH + NKV) * D + np.arange(2 * g2 * D, 2 * (g2 + 1) * D)
        krows = krows.reshape(2, 64)[:, _ILV].reshape(-1)
        rowsel = np.concatenate([qrows, krows, vrows])
        Wc = w_eff[rowsel]                          # (1280, 2880)
        bc = b_eff[rowsel]

        WcT = np.zeros((HIDP, 1280), np.float32)
        WcT[:HID] = Wc.T
        wqkv = _bf16(
            WcT.reshape(KT, 128, QKV_M, 128).transpose(2, 1, 0, 3).reshape(QKV_M, 128, KT * 128)
        )
        bqkv = np.ascontiguousarray(bc.reshape(QKV_M, 128).T)

        # attn feature f: tile ft=f//128, partition p=f%128 -> g=p//64, hq=ft
        f = np.arange(1024)
        colsel = (16 * g2 + 8 * ((f % 128) // 64) + f // 128) * D + (f % 64)
        WoT = np.zeros((1024, HIDP), np.float32)
        WoT[:, :HID] = out_w[:, colsel].T
        wout32 = (WoT.reshape(OUT_K, 128, OUT_M, 128)
                  .transpose(2, 1, 0, 3).reshape(OUT_M, 128, OUT_K * 128))
        wout = _bf16(wout32)
        # [p, m*256 + i*128 + f] = block (m, k=i) in fp8
        wo8 = np.ascontiguousarray(
            wout32[:, :, 0:256].transpose(1, 0, 2).reshape(128, OUT_M * 256)
            .astype(ml_dtypes.float8_e4m3))

        xp = np.zeros((HIDP, NT), np.float32)
        xp[:HID] = xn[b].T
        xt = _bf16(xp.reshape(KT, 128, NT).transpose(1, 0, 2).reshape(128, KT * NT))

        sinkrow = np.empty((1, 4 * CH), np.float32)
        for g in range(2):
            for a in range(2):
                hl = 8 * g + 4 * a + np.arange(4)        # local heads per quad
                se = np.exp(sinks[16 * g2 + hl].astype(np.float32))
                sinkrow[0, (2 * g + a) * CH:(2 * g + a + 1) * CH] = np.repeat(se, 128)

        wqc = np.empty((128, KT * 512), ml_dtypes.bfloat16)
        for j, mj in enumerate((8, 9, 0, 1)):
            for k in range(KT):
                wqc[:, k * 512 + j * 128:k * 512 + (j + 1) * 128] = \
                    wqkv[mj][:, k * 128:(k + 1) * 128]
        in_maps.append({
            "xt": xt, "wqkv": wqkv, "wqc": np.ascontiguousarray(wqc),
            "ones64": ones64, "maskdb": maskdb, "masklb": masklb,
            "wo8": wo8,
            "bqkv": bqkv, "wout": wout,
            "cos128": cos128, "sin128": sin128,
            "maskt": _round_f32r(maskt), "identb": _round_f32r(identb),
            "ident64": ident64,
            "sinkrow": _round_f32r(sinkrow), "sinksel": _round_f32r(sinksel),
        })
    return in_maps


def unshard(results, x, out_b):
    x = np.asarray(x, np.float32)
    out_b = np.asarray(out_b, np.float32)
    y = np.empty((B, SEQ, HID), np.float32)
    for b in range(B):
        acc = np.zeros((HIDP, NT), np.float64)
        for g2 in range(4):
            acc += results[4 * b + g2]["out_t"].reshape(HIDP, NT)
        y[b] = x[b] + acc[:HID].T.astype(np.float32) + out_b[None, :]
    return y


_NC_CACHE = []


def kernel(x, norm_w, qkv_w, qkv_b, out_w, out_b, sinks):
    in_maps = prep_inputs(x, norm_w, qkv_w, qkv_b, out_w, sinks)
    if not _NC_CACHE:
        _NC_CACHE.append(build_nc())
    nc = _NC_CACHE[0]
    res = run_bass_kernel_spmd(nc, in_maps, core_ids=list(range(8)))
    return unshard(res.results, x, out_b)


# revision 52
# speedup vs baseline: 1.0023x; 1.0023x over previous
"""Trainium2 Bass kernel: RMSNorm + QKV + YaRN RoPE + sliding-window GQA attention
with sink logits + output projection + residual.

Sharding: data-parallel over batch (2) x tensor-parallel over KV-head pairs (4).
Each of the 8 cores computes, for one batch element and 2 of the 8 KV heads
(16 of the 64 Q heads), the fused block and a partial output projection.
The host sums the 4 partial projections per batch and adds bias + residual.

Layout/precision strategy (v2):
  - RMSNorm fully folded on host: x is pre-normalized (fp32) then cast bf16.
  - Weights (qkv/out) and activations (x/q/k/v/attn) in bf16; matmul moving
    operands >=512 wide run at 1 cycle/row either way, so bf16 only halves
    DMA traffic, not PE time.  All PSUM accumulation stays fp32.
  - Softmax sink term folded into the PV accumulation as a 1-partition
    matmul; mask injected via matmul from a staged mask against identity.
  - Single merged schedule: K/V (+ Q m0) matmuls chase the x DMA stream,
    attention groups interleave with the Q m4..m7 tiles, output projection
    chunk 0 interleaves with the last attention groups.
"""

import numpy as np
import ml_dtypes

import concourse.bass as bass
import concourse.tile as tile
from concourse import bacc, mybir
from concourse.bass_utils import run_bass_kernel_spmd

# problem constants
B, SEQ, HID = 2, 1024, 2880
NH, NKV, D = 64, 8, 64
HIDP = 2944            # 23 * 128
KT = HIDP // 128       # 23 hidden k-tiles
QKV_M = 10             # 1280 rows per core / 128
OUT_M = KT             # output hidden tiles (padded)
OUT_K = 8              # 1024 attn features / 128
NT = SEQ               # tokens per core
CH = 512               # matmul moving chunk
EPS = 1e-5
MASK_NEG = -100.0

F32 = mybir.dt.float32
F32R = mybir.dt.float32r
BF16 = mybir.dt.bfloat16
FP8 = mybir.dt.float8e4

PAIR_SWAP = [i ^ 1 for i in range(32)]


# ---------------------------------------------------------------- device code
def build_nc(reps=1, timing_mode=False, debug=False):
    nc = bacc.Bacc("TRN2", target_bir_lowering=False, debug=False)

    big = "Internal" if timing_mode else "ExternalInput"
    xt_d = nc.dram_tensor("xt", [128, KT * NT], BF16, kind=big)
    wqkv_d = nc.dram_tensor("wqkv", [QKV_M, 128, KT * 128], BF16, kind=big)
    wqc_d = nc.dram_tensor("wqc", [128, KT * 512], BF16, kind=big)
    bqkv_d = nc.dram_tensor("bqkv", [128, QKV_M], F32, kind="ExternalInput")
    wout_d = nc.dram_tensor("wout", [OUT_M, 128, OUT_K * 128], BF16, kind=big)
    cos_d = nc.dram_tensor("cos128", [128, NT], BF16, kind="ExternalInput")
    sin_d = nc.dram_tensor("sin128", [128, NT], BF16, kind="ExternalInput")
    maskt_d = nc.dram_tensor("maskt", [128, 256], F32R, kind="ExternalInput")
    identb_d = nc.dram_tensor("identb", [128, CH], F32R, kind="ExternalInput")
    sink_d = nc.dram_tensor("sinkrow", [1, 4 * CH], F32R, kind="ExternalInput")
    sinksel_d = nc.dram_tensor("sinksel", [1, 65], F32R, kind="ExternalInput")
    id_d = nc.dram_tensor("ident64", [128, 64], BF16, kind="ExternalInput")
    ones64_d = nc.dram_tensor("ones64", [1, 64], F32R, kind="ExternalInput")
    wo8_d = nc.dram_tensor("wo8", [128, OUT_M * 256], FP8, kind=big)
    maskdb_d = nc.dram_tensor("maskdb", [128, CH], BF16, kind="ExternalInput")
    masklb_d = nc.dram_tensor("masklb", [128, CH], BF16, kind="ExternalInput")
    out_d = nc.dram_tensor("out_t", [OUT_M, 128, NT], F32,
                           kind="Internal" if timing_mode else "ExternalOutput")
    dummy_d = (nc.dram_tensor("timing_out", [1, 2], F32, kind="ExternalOutput")
               if timing_mode else None)
    if debug:
        dbg_k = nc.dram_tensor("dbg_k", [128, NT], BF16, kind="ExternalOutput")
        dbg_v = nc.dram_tensor("dbg_v", [128, NT], BF16, kind="ExternalOutput")
        dbg_q = nc.dram_tensor("dbg_q", [128, 8 * NT], BF16, kind="ExternalOutput")
        dbg_a = nc.dram_tensor("dbg_a", [128, 8 * NT], BF16, kind="ExternalOutput")
        dbg_vt = nc.dram_tensor("dbg_vt", [128, 16 * 65], BF16, kind="ExternalOutput")

    mult = mybir.AluOpType.mult

    with tile.TileContext(nc) as tc:
      for rep in range(reps):
        with tc.tile_pool(name="singles", bufs=1) as singles:
            cos_sb = singles.tile([128, NT], BF16)
            sin_sb = singles.tile([128, NT], BF16)
            bq_sb = singles.tile([128, QKV_M], F32)
            sink_sb = singles.tile([1, 4 * CH], F32R)
            sinksel_sb = singles.tile([1, 65], F32R)
            id_sb = singles.tile([128, 64], BF16)
            ones64_sb = singles.tile([1, 64], F32R)
            wo8_sb = singles.tile([128, OUT_M * 256], FP8)
            attn8_sb = singles.tile([128, 2 * NT], FP8)
            maskdb_sb = singles.tile([128, CH], BF16)
            masklb_sb = singles.tile([128, CH], BF16)
            maskt_sb = singles.tile([128, 256], F32R)
            identb_sb = singles.tile([128, CH], F32R)

            q_sb = singles.tile([128, 8 * NT], BF16)
            k_sb = singles.tile([128, NT], BF16)
            v_sb = singles.tile([128, NT], BF16)
            vt_sb = singles.tile([128, 16 * 65], BF16)

            q_v = q_sb.rearrange("p (h t) -> p h t", t=NT)

            with (
                tc.tile_pool(name="xtp", bufs=1) as xtp,
                tc.tile_pool(name="wqp", bufs=4) as wqp,
                tc.tile_pool(name="ropep", bufs=2) as ropep,
            ):
                xt_sb = xtp.tile([128, KT * NT], BF16)

                # --- DMA issue: x stream on sync queue; weights on pool queue
                # (kept off the Act queue so epilogues never delay loads).
                # wq8/wq9 in per-k chunks interleave with the x stream so the
                # K/V chase can start immediately; wq0 chunks ride along so
                # the Q m0 matmuls can join the chase.
                wqc_sb = xtp.tile([128, KT * 512], BF16)
                wq_tiles = {m: wqp.tile([128, KT * 128], BF16, tag="wq",
                                        name=f"wqm{m}")
                            for m in range(2, 8)}

                def wc_dma(lo, hi):
                    nc.sync.dma_start(out=wqc_sb[:, lo * 512:hi * 512],
                                      in_=wqc_d[:, lo * 512:hi * 512])

                def x_dma(lo, hi):
                    nc.sync.dma_start(out=xt_sb[:, lo * NT:hi * NT],
                                      in_=xt_d[:, lo * NT:hi * NT])

                nc.sync.dma_start(out=xt_sb[:, 0:CH], in_=xt_d[:, 0:CH])
                nc.sync.dma_start(out=wqc_sb[:, 0:512], in_=wqc_d[:, 0:512])
                nc.sync.dma_start(out=xt_sb[:, CH:NT], in_=xt_d[:, CH:NT])
                wc_dma(1, 4)
                x_dma(1, 2)
                x_dma(2, 5)
                wc_dma(4, 8)
                x_dma(5, 9)
                wc_dma(8, 13)
                def gate(sb, col):
                    # 1-element copy whose input arrives late: blocks the
                    # subsequent DMA write of sb from being hoisted early
                    nc.vector.tensor_copy(out=sb[0:1, 0:1],
                                          in_=xt_sb[0:1, col:col + 1])

                for sb in (cos_sb, sin_sb, bq_sb):
                    gate(sb, 8 * NT)
                nc.sync.dma_start(out=cos_sb, in_=cos_d[:, :])
                nc.sync.dma_start(out=sin_sb, in_=sin_d[:, :])
                nc.sync.dma_start(out=bq_sb, in_=bqkv_d[:, :])
                x_dma(9, 14)
                wc_dma(13, 18)
                x_dma(14, 19)
                wc_dma(18, KT)
                x_dma(19, KT)
                for sb in (sink_sb, sinksel_sb, maskt_sb, identb_sb, id_sb,
                           ones64_sb, maskdb_sb, masklb_sb):
                    gate(sb, 13 * NT)
                nc.sync.dma_start(out=sink_sb, in_=sink_d[:, :])
                nc.sync.dma_start(out=sinksel_sb, in_=sinksel_d[:, :])
                nc.sync.dma_start(out=maskt_sb, in_=maskt_d[:, :])
                nc.sync.dma_start(out=identb_sb, in_=identb_d[:, :])
                nc.sync.dma_start(out=id_sb, in_=id_d[:, :])
                nc.sync.dma_start(out=ones64_sb, in_=ones64_d[:, :])
                gate(wo8_sb, 13 * NT)
                nc.sync.dma_start(out=wo8_sb, in_=wo8_d[:, :])
                nc.sync.dma_start(out=maskdb_sb, in_=maskdb_d[:, :])
                nc.sync.dma_start(out=masklb_sb, in_=masklb_d[:, :])
                for m in range(2, 8):
                    gate(wq_tiles[m], (8 + m) * NT)
                    nc.sync.dma_start(out=wq_tiles[m], in_=wqkv_d[m, :, :])
                for t in range(16):
                    nc.vector.memset(vt_sb[:, t * 65 + 64:t * 65 + 65], 1.0)

                def qkv_dest(m):
                    if m == 8:
                        return k_sb
                    if m == 9:
                        return v_sb
                    return q_sb[:, m * NT:(m + 1) * NT]

                def qkv_epilogue(m, ps_cs):
                    dest = qkv_dest(m)
                    for c, ps in enumerate(ps_cs):
                        nc.scalar.activation(
                            out=dest[:, c * CH:(c + 1) * CH], in_=ps,
                            func=mybir.ActivationFunctionType.Identity,
                            bias=bq_sb[:, m:m + 1], scale=1.0,
                        )

                def rope(dest, c=None):
                    # per-chunk so the first 512 tokens unblock attention
                    # groups before the second half's DVE chain drains
                    cs = (0, 1) if c is None else (c,)
                    for cc in cs:
                        sl = slice(cc * CH, (cc + 1) * CH)
                        sh = ropep.tile([128, CH], BF16, tag="rope")
                        nc.vector.stream_shuffle(out=sh, in_=dest[:, sl],
                                                 mask=PAIR_SWAP)
                        nc.vector.tensor_mul(sh, sh, sin_sb[:, sl])
                        nc.vector.tensor_mul(dest[:, sl], dest[:, sl],
                                             cos_sb[:, sl])
                        nc.vector.tensor_add(dest[:, sl], dest[:, sl], sh)

                # --- chase: K/V and Q m0 matmuls follow the x stream
                kv_ctx = tc.tile_pool(name="ps_kv", bufs=2, space="PSUM")
                ps_kv = kv_ctx.__enter__()
                ps8 = ps_kv.tile([128, NT], F32, tag="kv", name="mm8")
                ps9 = ps_kv.tile([128, NT], F32, tag="kv", name="mm9")
                # warm-up matmuls on a memset tile keep the PE clock ramping
                # while the first x/weight DMAs land
                warm = xtp.tile([128, CH], BF16)
                nc.gpsimd.memset(warm, 0.0)
                for i in range(8):
                    nc.tensor.matmul(
                        ps8[:, 0:CH], warm[:, 0:128], warm,
                        start=(i == 0), stop=(i == 7),
                    )
                mm_ctx = tc.tile_pool(name="ps_mm", bufs=4, space="PSUM",
                                      side="right")
                ps_mm = mm_ctx.__enter__()
                ps0 = [ps_mm.tile([128, CH], F32, tag="mm", name=f"mm0c{c}")
                       for c in range(2)]
                ps1 = [ps_mm.tile([128, CH], F32, tag="mm", name=f"mm1c{c}")
                       for c in range(2)]
                for k in range(KT):
                    for j, psx in enumerate((ps8, ps9, ps0, ps1)):
                        lhsT = wqc_sb[:, k * 512 + j * 128:k * 512 + (j + 1) * 128]
                        for c in range(2):
                            out = (psx[:, c * CH:(c + 1) * CH] if j < 2
                                   else psx[c])
                            nc.tensor.matmul(
                                out,
                                lhsT,
                                xt_sb[:, k * NT + c * CH: k * NT + (c + 1) * CH],
                                start=(k == 0),
                                stop=(k == KT - 1),
                            )

                qkv_epilogue(0, ps0)
                rope(qkv_dest(0))
                qkv_epilogue(1, ps1)
                rope(qkv_dest(1))
                qkv_epilogue(8, (ps8[:, 0:CH], ps8[:, CH:NT]))
                rope(k_sb)
                qkv_epilogue(9, (ps9[:, 0:CH], ps9[:, CH:NT]))
                kv_ctx.__exit__(None, None, None)
                vt_ctx = tc.tile_pool(name="ps_vt", bufs=2, space="PSUM")
                ps_vt = vt_ctx.__enter__()
                for g in range(2):
                    for kt in range(8):
                        pst = ps_vt.tile([128, 64], BF16, tag="vt")
                        nc.tensor.matmul(
                            pst,
                            v_sb[g * 64:(g + 1) * 64, kt * 128:(kt + 1) * 128],
                            id_sb[g * 64:(g + 1) * 64, :],
                            is_transpose=True,
                            start=True, stop=True,
                        )
                        nc.vector.tensor_copy(
                            out=vt_sb[:, (g * 8 + kt) * 65:(g * 8 + kt) * 65 + 64],
                            in_=pst,
                        )
                vt_ctx.__exit__(None, None, None)

                def qkv_mms(m, wqx, inject=False):
                    ps_cs = []
                    for c in range(2):
                        ps = ps_mm.tile([128, CH], F32, tag="mm",
                                        name=f"mm{m}c{c}")
                        ps_cs.append(ps)
                        for k in range(KT):
                            lhsT = wqx[:, k * 128:(k + 1) * 128]
                            nc.tensor.matmul(
                                ps,
                                lhsT,
                                xt_sb[:, k * NT + c * CH: k * NT + (c + 1) * CH],
                                start=(k == 0),
                                stop=(k == KT - 1),
                            )
                            if inject and k % 4 == 3:
                                att_step(copy_mask=True)
                    return ps_cs

                def q_tile(m, inject=False):
                    ps_cs = qkv_mms(m, wq_tiles.pop(m), inject)
                    dest = qkv_dest(m)
                    for c, ps in enumerate(ps_cs):
                        nc.scalar.activation(
                            out=dest[:, c * CH:(c + 1) * CH], in_=ps,
                            func=mybir.ActivationFunctionType.Identity,
                            bias=bq_sb[:, m:m + 1], scale=1.0,
                        )
                        rope(dest, c)

                for m in range(2, 4):
                    q_tile(m)

                # --- attention machinery (shares the window with Q m4..m7)
                att_ctx = [
                    tc.tile_pool(name="attnp", bufs=1),
                    tc.tile_pool(name="wexp", bufs=5),
                    tc.tile_pool(name="dnp", bufs=6),
                ]
                attnp, wexp, dnp = [c.__enter__() for c in att_ctx]
                att1_ctx = [
                    tc.tile_pool(name="ps_att", bufs=2, space="PSUM"),
                    tc.tile_pool(name="ps_pv", bufs=2, space="PSUM"),
                ]
                cur_pools = [[c.__enter__() for c in att1_ctx] + [None]]

                attn_sb = attnp.tile([128, 8 * NT], BF16)
                a_v = attn_sb.rearrange("p (h t) -> p h t", t=NT)
                a8_v = attn8_sb.rearrange("p (h t) -> p h t", t=NT)
                wo8_v = wo8_sb.rearrange("p (m two f) -> p m two f",
                                         two=2, f=128)

                def attn_A(qt, a, g, copy_mask=False):
                    # copy_mask=True: skip the additive mask matmul; apply a
                    # binary mask multiplicatively on the exp output (DVE,
                    # bf16 2x).  Masked logits share the live logits' scale,
                    # so exp cannot overflow.
                    ps_att, ps_pv = cur_pools[0][:2]
                    prng = slice(g * 64, (g + 1) * 64)
                    kts = [qt] if qt == 0 else [qt - 1, qt]
                    rhs_q = q_v[prng, 4 * a:4 * a + 4, qt * 128:(qt + 1) * 128]
                    ws = []
                    for kt in kts:
                        psl = ps_att.tile([128, CH], F32, tag="l")
                        sel = 0 if kt == qt else 1
                        if copy_mask:
                            nc.tensor.matmul(
                                psl,
                                k_sb[prng, kt * 128:(kt + 1) * 128],
                                rhs_q,
                                start=True, stop=True,
                            )
                        else:
                            nc.tensor.matmul(
                                psl,
                                maskt_sb[:, sel * 128:(sel + 1) * 128],
                                identb_sb,
                                start=True, stop=False,
                            )
                            nc.tensor.matmul(
                                psl,
                                k_sb[prng, kt * 128:(kt + 1) * 128],
                                rhs_q,
                                start=False, stop=True,
                            )
                        w = wexp.tile([128, CH], BF16, tag="w")
                        nc.scalar.activation(
                            out=w, in_=psl, func=mybir.ActivationFunctionType.Exp
                        )
                        if copy_mask:
                            nc.vector.tensor_mul(
                                w, w, maskdb_sb if sel == 0 else masklb_sb
                            )
                        ws.append((kt, w))
                    return (qt, a, g, ws)

                def attn_B(st, pe_bcast=False):
                    ps_att, ps_pv = cur_pools[0][:2]
                    qt, a, g, ws = st
                    prng = slice(g * 64, (g + 1) * 64)
                    pspv = ps_pv.tile([65, CH], F32, tag="pv")
                    so = (2 * g + a) * CH
                    nc.tensor.matmul(
                        pspv, sinksel_sb, sink_sb[0:1, so:so + CH],
                        start=True, stop=False,
                    )
                    for i, (kt, w) in enumerate(ws):
                        nc.tensor.matmul(
                            pspv,
                            vt_sb[:, (g * 8 + kt) * 65:(g * 8 + kt + 1) * 65],
                            w,
                            start=False,
                            stop=(i == len(ws) - 1),
                        )
                    dn = dnp.tile([1, CH], F32, tag="dn")
                    dnb = dnp.tile([64, CH], F32, tag="dnb")
                    if pe_bcast:
                        # half-width chains (head pairs) pipeline across
                        # DVE/Pool, halving the latency that gates the pspv
                        # ring in attention-dense windows
                        for h in range(2):
                            sl = slice(h * 256, (h + 1) * 256)
                            nc.vector.reciprocal(out=dn[0:1, sl],
                                                 in_=pspv[64:65, sl])
                            nc.gpsimd.partition_broadcast(dnb[:, sl],
                                                          dn[0:1, sl])
                            nc.vector.tensor_tensor(
                                out=a_v[prng, 4 * a + 2 * h:4 * a + 2 * h + 2,
                                        qt * 128:(qt + 1) * 128],
                                in0=pspv[0:64, sl],
                                in1=dnb[:, sl],
                                op=mult,
                            )
                    else:
                        nc.vector.reciprocal(out=dn, in_=pspv[64:65, :])
                        nc.gpsimd.partition_broadcast(dnb, dn)
                        nc.vector.tensor_tensor(
                            out=a_v[prng, 4 * a:4 * a + 4,
                                    qt * 128:(qt + 1) * 128],
                            in0=pspv[0:64, :],
                            in1=dnb,
                            op=mult,
                        )
                    if a == 0:
                        # heads 0-1 also in fp8 for the DoubleRow pair
                        nc.vector.tensor_copy(
                            out=a8_v[prng, 0:2, qt * 128:(qt + 1) * 128],
                            in_=a_v[prng, 0:2, qt * 128:(qt + 1) * 128],
                        )

                from collections import deque
                att_pending = deque()
                att_inflight = deque()

                def att_step(copy_mask=False, pe_bcast=False):
                    if len(att_inflight) >= 2 or (not att_pending
                                                  and att_inflight):
                        attn_B(att_inflight.popleft(), pe_bcast=pe_bcast)
                    elif att_pending:
                        att_inflight.append(
                            attn_A(*att_pending.popleft(),
                                   copy_mask=copy_mask))

                def att_drain():
                    while att_pending or att_inflight:
                        att_step()

                def attn_group(qt, a, g):
                    att_pending.append((qt, a, g))
                    att_step()
                    att_step()

                # W1: Q m4..m7 with ALL a=0 attention groups pipelined;
                # the a=1 qt0-3 groups queue up behind them (their q tiles
                # finish during W1)
                att_pending.extend((qt, 0, g) for qt in range(8)
                                   for g in range(2))
                for m in range(4, 8):
                    q_tile(m, inject=True)
                att_drain()
                att_pending.extend((qt, 1, g) for qt in range(4)
                                   for g in range(2))
                for c in reversed(att1_ctx):
                    c.__exit__(None, None, None)
                mm_ctx.__exit__(None, None, None)
                att2_ctx = [
                    tc.tile_pool(name="ps_att2", bufs=3, space="PSUM"),
                    tc.tile_pool(name="ps_pv2", bufs=3, space="PSUM"),
                ]
                cur_pools[0] = [c.__enter__() for c in att2_ctx]

                # W2: a=1 qt0-3, pipelined (must fully drain before the
                # chunk-0 projection reads their outputs: issue order defines
                # read/write ordering for the dependency pass)
                for _ in range(16):
                    att_step(pe_bcast=True)
                att_drain()

                # W3/W4: output projection; chunk 0 interleaves with the
                # remaining attention groups (a=1, qt 4..7)
                with (
                    tc.tile_pool(name="wop", bufs=3) as wop,
                    tc.tile_pool(name="otp", bufs=3) as otp,
                    tc.tile_pool(name="ps_o", bufs=2, space="PSUM") as ps_o,
                ):
                    def outproj_m(cs, m, inject=False):
                        wo_sb = wop.tile([128, (OUT_K - 2) * 128], BF16,
                                         tag="wo", name=f"wo{cs[0]}_{m}")
                        if cs[0] == 0 and m < 3:
                            nc.vector.tensor_copy(out=wo_sb[0:1, 0:1],
                                                  in_=attn_sb[0:1, 0:1])
                        nc.scalar.dma_start(out=wo_sb,
                                            in_=wout_d[m, :, 256:OUT_K * 128])
                        for c in cs:
                            ps = ps_o.tile([128, CH], F32, tag="o",
                                           name=f"o{c}_{m}")
                            nc.tensor.matmul(
                                ps,
                                wo8_v[:, m, :, :],
                                a8_v[:, 0:2, c * CH:(c + 1) * CH],
                                start=True, stop=False,
                                perf_mode=mybir.MatmulPerfMode.DoubleRow,
                            )
                            for k in range(2, OUT_K):
                                nc.tensor.matmul(
                                    ps,
                                    wo_sb[:, (k - 2) * 128:(k - 1) * 128],
                                    attn_sb[:, k * NT + c * CH: k * NT + (c + 1) * CH],
                                    start=False,
                                    stop=(k == OUT_K - 1),
                                )
                                if inject and k % 3 == 2:
                                    att_step(copy_mask=True, pe_bcast=True)
                            ot = otp.tile([128, CH], F32, tag="ot")
                            nc.scalar.copy(out=ot, in_=ps)
                            nc.sync.dma_start(
                                out=out_d[m, :, c * CH:(c + 1) * CH],
                                in_=ot,
                            )

                    # m 0..7: chunk-0 projection with the last attention
                    # groups pipelined in; m 8..22: both chunks per wo load;
                    # then chunk 1 for m 0..7 (wo reloaded).
                    att_pending.extend((qt, 1, g) for qt in range(4, 8)
                                       for g in range(2))
                    for m in range(8):
                        outproj_m((0,), m, inject=True)
                    att_drain()

                    for m in range(8, OUT_M):
                        outproj_m((0, 1), m)
                    for m in range(7):
                        outproj_m((1,), m)
                    # final tile: two half-width PSUM chains so the first
                    # half's copy+store overlap the second half's matmuls
                    wo_sb = wop.tile([128, (OUT_K - 2) * 128], BF16,
                                     tag="wo", name="wo1_7f")
                    nc.scalar.dma_start(out=wo_sb,
                                        in_=wout_d[7, :, 256:OUT_K * 128])
                    for h, q in ((0, nc.scalar), (1, nc.sync)):
                        psf = ps_o.tile([128, CH], F32, tag="o", name=f"oh{h}")
                        ps = psf[:, 0:CH // 2]
                        nc.tensor.matmul(
                            ps,
                            wo8_v[:, 7, :, :],
                            a8_v[:, 0:2, CH + h * 256:CH + (h + 1) * 256],
                            start=True, stop=False,
                            perf_mode=mybir.MatmulPerfMode.DoubleRow,
                        )
                        for k in range(2, OUT_K):
                            nc.tensor.matmul(
                                ps,
                                wo_sb[:, (k - 2) * 128:(k - 1) * 128],
                                attn_sb[:, k * NT + CH + h * 256:
                                        k * NT + CH + (h + 1) * 256],
                                start=False,
                                stop=(k == OUT_K - 1),
                            )
                        ot = otp.tile([128, CH // 2], F32, tag="oth")
                        if h == 0:
                            nc.scalar.copy(out=ot, in_=ps)
                        else:
                            nc.vector.tensor_copy(out=ot, in_=ps)
                        q.dma_start(
                            out=out_d[7, :, CH + h * 256:CH + (h + 1) * 256],
                            in_=ot,
                        )

                if debug:
                    for dram, sb in ((dbg_k, k_sb), (dbg_v, v_sb),
                                     (dbg_q, q_sb), (dbg_a, attn_sb),
                                     (dbg_vt, vt_sb)):
                        nc.sync.dma_start(out=dram[:, :], in_=sb)

                for c in reversed(att2_ctx):
                    c.__exit__(None, None, None)
                for c in reversed(att_ctx):
                    c.__exit__(None, None, None)

      if timing_mode:
          with tc.tile_pool(name="dummyp", bufs=1) as dummyp:
              dt_sb = dummyp.tile([1, 2], F32)
              nc.vector.memset(dt_sb, 1.0)
              nc.sync.dma_start(out=dummy_d[:, :], in_=dt_sb)

    nc.compile()
    return nc


# ---------------------------------------------------------------- host prep
def _rope_tables():
    # verbatim fp32 port of the reference YaRN cache
    steps = np.arange(0, 64, 2, dtype=np.float32)
    freq = np.power(np.float32(150000.0), steps / np.float32(64))
    conc = np.float32(0.1) * np.log(np.float32(32.0)) + 1.0
    d_half = np.float32(32.0)
    log_base = np.log(np.float32(150000.0))
    low = d_half * np.log(np.float32(4096) / (np.float32(32.0) * np.float32(2.0 * np.pi))) / log_base
    high = d_half * np.log(np.float32(4096) / (np.float32(1.0) * np.float32(2.0 * np.pi))) / log_base
    ramp = (np.arange(32, dtype=np.float32) - low) / (high - low)
    mask = 1.0 - np.clip(ramp, 0.0, 1.0)
    inv_freq = (1.0 / (np.float32(32.0) * freq)) * (1.0 - mask) + (1.0 / freq) * mask
    pos = np.arange(SEQ, dtype=np.float32)
    freqs = np.einsum("i,j->ij", pos, inv_freq.astype(np.float32))
    cos = (np.cos(freqs) * conc).astype(np.float32)  # (SEQ, 32)
    sin = (np.sin(freqs) * conc).astype(np.float32)
    return cos, sin


def _round_f32r(a):
    """Round-to-nearest onto the fp32r grid (low 12 mantissa bits zero)."""
    b = np.ascontiguousarray(a, np.float32).view(np.uint32)
    out = ((b + 0x800) & 0xFFFFF000).astype(np.uint32)
    return out.view(np.float32)


def _bf16(a):
    return np.ascontiguousarray(a.astype(ml_dtypes.bfloat16))


_ILV = np.empty(64, np.int64)
_ILV[0::2] = np.arange(32)
_ILV[1::2] = np.arange(32) + 32


def prep_inputs(x, norm_w, qkv_w, qkv_b, out_w, sinks):
    x = np.asarray(x, np.float32)
    norm_w = np.asarray(norm_w, np.float32)
    qkv_w = np.asarray(qkv_w, np.float32)
    qkv_b = np.asarray(qkv_b, np.float32)
    out_w = np.asarray(out_w, np.float32)
    sinks = np.asarray(sinks, np.float32)

    # host RMSNorm (norm_w folded into qkv_w below)
    rms = np.mean(x * x, axis=-1, keepdims=True, dtype=np.float32)
    xn = x * (1.0 / np.sqrt(rms + np.float32(EPS)))

    cos, sin = _rope_tables()
    cosT, sinT = cos.T, sin.T                      # (32, SEQ)
    cos64 = np.repeat(cosT, 2, axis=0)             # lo/hi both use cos_i
    sin64 = np.repeat(sinT, 2, axis=0).copy()
    sin64[0::2] *= -1.0                            # lo gets -sin
    cos128 = _bf16(np.concatenate([cos64, cos64], axis=0))
    sin128 = _bf16(np.concatenate([sin64, sin64], axis=0))

    i = np.arange(128)[:, None]
    j = np.arange(128)[None, :]
    maskd = np.where(i <= j, 0.0, MASK_NEG).astype(np.float32)
    maskl = np.where(i > j, 0.0, MASK_NEG).astype(np.float32)
    maskt = np.ascontiguousarray(np.concatenate([maskd.T, maskl.T], axis=1))
    identb = np.ascontiguousarray(np.tile(np.eye(128, dtype=np.float32), (1, 4)))
    sinksel = np.zeros((1, 65), np.float32)
    sinksel[0, 64] = 1.0
    eye = np.eye(64, dtype=np.float32)
    ident64 = _bf16(np.concatenate([eye, eye], axis=0))  # (128, 64)
    ones64 = np.ones((1, 64), np.float32)
    bind = np.where(i <= j, 1.0, 0.0).astype(np.float32)   # keep: diag block
    binl = np.where(i > j, 1.0, 0.0).astype(np.float32)    # keep: low block
    maskdb = _bf16(np.tile(bind, (1, 4)))
    masklb = _bf16(np.tile(binl, (1, 4)))

    w_eff = qkv_w * norm_w[None, :]
    b_eff = qkv_b.copy()
    w_eff[:NH * D] *= 0.125
    b_eff[:NH * D] *= 0.125

    in_maps = []
    for c in range(8):
        b, g2 = divmod(c, 4)
        # Q m-tile m holds heads (16*g2+m) [partitions 0:64] and (16*g2+8+m)
        # [partitions 64:128], rope-pair interleaved within each head.
        qheads = np.empty(16, np.int64)
        qheads[0::2] = 16 * g2 + np.arange(8)        # g=0 heads, even slots
        qheads[1::2] = 16 * g2 + 8 + np.arange(8)    # g=1 heads, odd slots
        qrows = (qheads[:, None] * D + _ILV[None, :]).reshape(-1)
        krows = NH * D + np.arange(2 * g2 * D, 2 * (g2 + 1) * D)
        vrows = (NH + NKV) * D + np.arange(2 * g2 * D, 2 * (g2 + 1) * D)
        krows = krows.reshape(2, 64)[:, _ILV].reshape(-1)
        rowsel = np.concatenate([qrows, krows, vrows])
        Wc = w_eff[rowsel]                          # (1280, 2880)
        bc = b_eff[rowsel]

        WcT = np.zeros((HIDP, 1280), np.float32)
        WcT[:HID] = Wc.T
        wqkv = _bf16(
            WcT.reshape(KT, 128, QKV_M, 128).transpose(2, 1, 0, 3).reshape(QKV_M, 128, KT * 128)
        )
        bqkv = np.ascontiguousarray(bc.reshape(QKV_M, 128).T)

        # attn feature f: tile ft=f//128, partition p=f%128 -> g=p//64, hq=ft
        f = np.arange(1024)
        colsel = (16 * g2 + 8 * ((f % 128) // 64) + f // 128) * D + (f % 64)
        WoT = np.zeros((1024, HIDP), np.float32)
        WoT[:, :HID] = out_w[:, colsel].T
        wout32 = (WoT.reshape(OUT_K, 128, OUT_M, 128)
                  .transpose(2, 1, 0, 3).reshape(OUT_M, 128, OUT_K * 128))
        wout = _bf16(wout32)
        # [p, m*256 + i*128 + f] = block (m, k=i) in fp8
        wo8 = np.ascontiguousarray(
            wout32[:, :, 0:256].transpose(1, 0, 2).reshape(128, OUT_M * 256)
            .astype(ml_dtypes.float8_e4m3))

        xp = np.zeros((HIDP, NT), np.float32)
        xp[:HID] = xn[b].T
        xt = _bf16(xp.reshape(KT, 128, NT).transpose(1, 0, 2).reshape(128, KT * NT))

        sinkrow = np.empty((1, 4 * CH), np.float32)
        for g in range(2):
            for a in range(2):
                hl = 8 * g + 4 * a + np.arange(4)        # local heads per quad
                se = np.exp(sinks[16 * g2 + hl].astype(np.float32))
                sinkrow[0, (2 * g + a) * CH:(2 * g + a + 1) * CH] = np.repeat(se, 128)

        wqc = np.empty((128, KT * 512), ml_dtypes.bfloat16)
        for j, mj in enumerate((8, 9, 0, 1)):
            for k in range(KT):
                wqc[:, k * 512 + j * 128:k * 512 + (j + 1) * 128] = \
                    wqkv[mj][:, k * 128:(k + 1) * 128]
        in_maps.append({
            "xt": xt, "wqkv": wqkv, "wqc": np.ascontiguousarray(wqc),
            "ones64": ones64, "maskdb": maskdb, "masklb": masklb,
            "wo8": wo8,
            "bqkv": bqkv, "wout": wout,
            "cos128": cos128, "sin128": sin128,
            "maskt": _round_f32r(maskt), "identb": _round_f32r(identb),
            "ident64": ident64,
            "sinkrow": _round_f32r(sinkrow), "sinksel": _round_f32r(sinksel),
        })
    return in_maps


def unshard(results, x, out_b):
    x = np.asarray(x, np.float32)
    out_b = np.asarray(out_b, np.float32)
    y = np.empty((B, SEQ, HID), np.float32)
    for b in range(B):
        acc = np.zeros((HIDP, NT), np.float64)
        for g2 in range(4):
            acc += results[4 * b + g2]["out_t"].reshape(HIDP, NT)
        y[b] = x[b] + acc[:HID].T.astype(np.float32) + out_b[None, :]
    return y


_NC_CACHE = []


def kernel(x, norm_w, qkv_w, qkv_b, out_w, out_b, sinks):
    in_maps = prep_inputs(x, norm_w, qkv_w, qkv_b, out_w, sinks)
    if not _NC_CACHE:
        _NC_CACHE.append(build_nc())
    nc = _NC_CACHE[0]
    res = run_bass_kernel_spmd(nc, in_maps, core_ids=list(range(8)))
    return unshard(res.results, x, out_b)
